# revision 23
# baseline (speedup 1.0000x reference)
"""CBAM kernel for Trainium2, 8-way batch-parallel SPMD.

Computes out = x^2 * (att_c[b,c] + sigmoid(conv(spatial_stats))[b,l]) where
att_c = sigmoid(mlp(mean_L x) + mlp(max_L x)), matching the CBAM reference.

Layout per core: 4 batches; each batch x[4096, 256] lives in SBUF as one
[128, 8192] bf16 tensor (partition p holds DRAM rows [32p, 32p+32), i.e.
32 KiB contiguous fp32 DRAM per partition; the fp32->bf16 cast happens for
free inside the SWDGE load DMA). l = 32*p + j, free col = 256*j + c.

Engine split per batch (~23 us DMA window):
  DVE  : channel-max + spatial-max bf16 fold trees (2x perf mode), 8-j
         spatial-sum tree, the 32 final (att+sig)*x^2 STTs (deferred one
         batch so they fill the next load window), small reduces
  ACT  : 24-j spatial-sum accumulator copies, squares (bf16), sigmoids,
         PSUM->SBUF copies
  PE   : channel-sum (16x [128,512] bf16 matmuls), stats transposes, MLP,
         7-tap conv as banded-Toeplitz matmuls in transposed [j, p] space
         (corner taps = column-shifted rhs, no halo exchange needed)
  POOL : SWDGE DMA queue only - cast loads, cast stores (bf16 -> fp32)
"""

import numpy as np
from contextlib import ExitStack

import concourse.bacc as bacc
import concourse.bass as bass
import concourse.tile as tile
import concourse.mybir as mybir
from concourse.bass_utils import run_bass_kernel_spmd

AF = mybir.ActivationFunctionType
ALU = mybir.AluOpType
AX = mybir.AxisListType
FP32 = mybir.dt.float32
BF16 = mybir.dt.bfloat16

N_CORES = 8
B_FULL = 32
NB = B_FULL // N_CORES  # batches per core = 4
L = 4096
C = 256
HID = 16
HB = HID + 1
P = 128
NJ = L // P  # 32 j-blocks (rows per partition)
HJ = NJ // 2  # 16 j-blocks per half
HALF = HJ * C  # 4096 free columns per half

_CACHE: dict = {}


def _build_body(ctx: ExitStack, tc, out_d, x_d, w1_d, b1_d, w2b_d, id_d,
                id16_d, ones_d, rc16_d, tj_d, reps=1):
    nc = tc.nc

    const = ctx.enter_context(tc.tile_pool(name="const", bufs=1))
    bpool = ctx.enter_context(tc.tile_pool(name="xb16", bufs=3))
    opool = ctx.enter_context(tc.tile_pool(name="outt", bufs=2))
    sqpool = ctx.enter_context(tc.tile_pool(name="sq", bufs=4))
    mpool = ctx.enter_context(tc.tile_pool(name="maxtree", bufs=1))
    smpool = ctx.enter_context(tc.tile_pool(name="spattree", bufs=1))
    spool = ctx.enter_context(tc.tile_pool(name="stats", bufs=2))
    dpool = ctx.enter_context(tc.tile_pool(name="dummy", bufs=2))
    pacc = ctx.enter_context(tc.tile_pool(name="pacc", bufs=2, space="PSUM"))
    pwk = ctx.enter_context(tc.tile_pool(name="pwk", bufs=2, space="PSUM"))
    pcnv = ctx.enter_context(tc.tile_pool(name="pcnv", bufs=2, space="PSUM"))
    pb16 = ctx.enter_context(tc.tile_pool(name="pb16", bufs=2, space="PSUM"))

    w1 = const.tile([P, 2 * HB], FP32)
    nc.sync.dma_start(w1[:], w1_d[:])
    b1 = const.tile([HB, 1], FP32)
    nc.sync.dma_start(b1[:], b1_d[:])
    w2b = const.tile([HB, C], FP32)
    nc.sync.dma_start(w2b[:], w2b_d[:])
    ident = const.tile([P, P], FP32)
    nc.sync.dma_start(ident[:], id_d[:])
    ident16 = const.tile([P, P], BF16)
    nc.sync.dma_start(ident16[:], id16_d[:])
    ones = const.tile([P, P], FP32)
    nc.sync.dma_start(ones[:], ones_d[:])
    redcol16 = const.tile([P, 1], BF16)
    nc.sync.dma_start(redcol16[:], rc16_d[:])
    tj = const.tile([NJ, 6 * NJ], FP32)
    nc.sync.dma_start(tj[:], tj_d[:])

    NSA = 20  # j-blocks whose spatial sum rides ACT accumulator copies

    def emit_final(prev):
        """Final combine (att + sig) * x^2 for the previous batch + stores.

        Emitted at the top of the next iteration so the 32 DVE STTs fill
        the load window of the current batch, and the stores land on the
        SWDGE queue right behind the current batch's loads."""
        att, psig, sqs, pb = prev
        ot = opool.tile([P, NJ * C], FP32, tag="ot", name="ot")
        for j in range(NJ):
            jh = j % HJ
            nc.vector.scalar_tensor_tensor(ot[:, C * j:C * (j + 1)],
                                           att[:], psig[:, j:j + 1],
                                           sqs[j // HJ][:, C * jh:C * (jh + 1)],
                                           op0=ALU.add, op1=ALU.mult)
        ov = out_d[pb, :, :].rearrange("(p q) c -> p (q c)", p=P)
        QW = HALF // 2
        for q4 in range(4):
            nc.sync.dma_start(ov[:, QW * q4:QW * (q4 + 1)],
                              ot[:, QW * q4:QW * (q4 + 1)])

    prev = None
    seq = [b for _ in range(reps) for b in range(NB)]
    for it, b in enumerate(seq):
        last = it == len(seq) - 1
        xb = bpool.tile([P, NJ * C], BF16, tag="xb", name="xb")
        xv = x_d[b, :, :].rearrange("(p q) c -> p (q c)", p=P)
        QW = HALF // 2
        for q4 in range(4):
            nc.gpsimd.dma_start(xb[:, QW * q4:QW * (q4 + 1)],
                                xv[:, QW * q4:QW * (q4 + 1)])

        # On the last iteration the current batch's stats chain is the
        # critical path to the final stores - emit it before the previous
        # batch's (already-gated) combine so DVE prioritises it.
        if prev is not None and not last:
            emit_final(prev)
        lastprev = prev if last else None

        sej = spool.tile([P, NJ], FP32, tag="sej", name="sej")
        semf = spool.tile([P, NJ], FP32, tag="semf", name="semf")
        pcs = pacc.tile([1, 2 * C], FP32, tag="pcs")
        sqs = []
        for h in range(2):
            # ---- ACT: spatial-sum accumulator copies (j < NSA) ----
            for jh in range(HJ):
                j = HJ * h + jh
                if j < NSA:
                    dummy = dpool.tile([P, C], BF16, tag="dummy")
                    nc.scalar.activation(dummy[:], xb[:, C * j:C * (j + 1)],
                                         AF.Identity,
                                         accum_out=sej[:, j:j + 1])
            # ---- PE: channel sum, 8x [128, 512] bf16 matmuls per half ----
            for m8 in range(8):
                m = 8 * h + m8
                nc.tensor.matmul(pcs[:], redcol16[:],
                                 xb[:, 512 * m:512 * (m + 1)],
                                 start=(m == 0), stop=(m == 15),
                                 skip_group_check=True)

        # ---- DVE: channel max over j, flat bf16 fold tree ----
        mh = mpool.tile([P, NJ * C // 2], BF16, tag="mh", name="mh")
        nc.vector.tensor_max(mh[:], xb[:, 0:HALF], xb[:, HALF:2 * HALF])
        w = HALF // 2
        while w >= C:
            nc.vector.tensor_max(mh[:, 0:w], mh[:, 0:w], mh[:, w:2 * w])
            w //= 2

        # ---- DVE: spatial max over c, strided bf16 fold tree ----
        sm = smpool.tile([P, NJ * (C // 2)], BF16, tag="sm", name="sm")
        sm3 = sm[:].rearrange("p (j c) -> p j c", c=C // 2)
        v3 = xb[:].rearrange("p (j c) -> p j c", c=C)
        nc.vector.tensor_max(sm3[:, :, :], v3[:, :, 0:C // 2],
                             v3[:, :, C // 2:C])
        w = C // 4
        while w >= 1:
            out = (semf[:, :].rearrange("p (j o) -> p j o", o=1)
                   if w == 1 else sm3[:, :, 0:w])
            nc.vector.tensor_max(out, sm3[:, :, 0:w], sm3[:, :, w:2 * w])
            w //= 2

        # ---- DVE: spatial sum for j >= NSA, small bf16 add tree ----
        nrem = NJ - NSA
        ss = smpool.tile([P, nrem * (C // 2)], BF16, tag="ss", name="ss")
        ss3 = ss[:].rearrange("p (j c) -> p j c", c=C // 2)
        v8 = xb[:, C * NSA:C * NJ].rearrange("p (j c) -> p j c", c=C)
        nc.vector.tensor_add(ss3[:, :, :], v8[:, :, 0:C // 2],
                             v8[:, :, C // 2:C])
        w = C // 4
        while w >= 1:
            out = (sej[:, NSA:NJ].rearrange("p (j o) -> p j o", o=1)
                   if w == 1 else ss3[:, :, 0:w])
            nc.vector.tensor_add(out, ss3[:, :, 0:w], ss3[:, :, w:2 * w])
            w //= 2

        # ---- channel stats into c-major [128, 4] via PE transposes ----
        avgw = spool.tile([1, 2 * C], FP32, tag="avgw", name="avgw")
        nc.scalar.copy(avgw[:], pcs[0:1, :])
        avg_row = spool.tile([1, C], FP32, tag="avg", name="avg")
        nc.vector.tensor_add(avg_row[:], avgw[0:1, 0:C], avgw[0:1, C:2 * C])
        stats = spool.tile([P, 4], FP32, tag="stats", name="stats")
        # fp32 PSUM bank: po 0:256, ph 256:258, pT 258:260
        wk = pwk.tile([P, C + 4], FP32, tag="wk")
        pT = wk[:, C + 2:C + 4]
        nc.tensor.transpose(pT[:, 0:1], avg_row[0:1, 0:P], ident[0:1, 0:1])
        nc.tensor.transpose(pT[:, 1:2], avg_row[0:1, P:C], ident[0:1, 0:1])
        # bf16 PSUM bank: mT 0:256, sig 256:288
        mtp = pb16.tile([P, 2 * P + NJ], BF16, tag="mtp")
        mT = mtp[:, 0:2 * P]
        nc.tensor.transpose(mT[:, 0:P], mh[:, 0:P], ident16[:, :])
        nc.tensor.transpose(mT[:, P:2 * P], mh[:, P:C], ident16[:, :])
        nc.scalar.copy(stats[:, 0:1], pT[:, 0:1])
        nc.scalar.copy(stats[:, 2:3], pT[:, 1:2])
        nc.vector.tensor_reduce(stats[:, 1:2], mT[:, 0:P],
                                axis=AX.X, op=ALU.max)
        nc.vector.tensor_reduce(stats[:, 3:4], mT[:, P:2 * P],
                                axis=AX.X, op=ALU.max)

        # ---- shared MLP: row HID carries the 2*b2 constant trick ----
        ph = wk[0:HB, C:C + 2]
        nc.tensor.matmul(ph[:], w1[:, 0:HB], stats[:, 0:2],
                         start=True, stop=False, skip_group_check=True)
        nc.tensor.matmul(ph[:], w1[:, HB:2 * HB], stats[:, 2:4],
                         start=False, stop=True, skip_group_check=True)
        hsb = spool.tile([HB, 2], FP32, tag="hsb", name="hsb")
        nc.scalar.activation(hsb[:], ph[:], AF.Relu, bias=b1[:])
        h2 = spool.tile([HB, 1], FP32, tag="h2", name="h2")
        nc.vector.tensor_add(h2[:], hsb[:, 0:1], hsb[:, 1:2])
        h2r = spool.tile([HB, P], FP32, tag="h2r", name="h2r")
        nc.scalar.mul(h2r[:], ones[0:HB, :], h2[:])
        po = wk[:, 0:C]
        nc.tensor.matmul(po[:], h2r[:], w2b[:], start=True, stop=True,
                         skip_group_check=True)
        att = spool.tile([P, C], BF16, tag="att", name="att")
        nc.scalar.activation(att[:], po[:], AF.Sigmoid)

        # ---- 7-tap conv in transposed [j, p] space (PE Toeplitz) ----
        # fp32 PSUM bank: sjT 0:128, smT 128:256, pcv 256:384
        pct = pcnv.tile([NJ, 3 * P], FP32, tag="pct")
        nc.tensor.transpose(pct[:, 0:P], sej[:], ident[:, :])
        nc.tensor.transpose(pct[:, P:2 * P], semf[:], ident[:, :])
        sjS = spool.tile([NJ, P], FP32, tag="sjS", name="sjS")
        nc.scalar.copy(sjS[:], pct[:, 0:P])
        smS = spool.tile([NJ, P], FP32, tag="smS", name="smS")
        nc.scalar.copy(smS[:], pct[:, P:2 * P])
        pcv = pct[:, 2 * P:3 * P]
        nc.tensor.matmul(pcv[:, :], tj[:, 0:NJ], sjS[:, :],
                         start=True, stop=False, skip_group_check=True)
        nc.tensor.matmul(pcv[:, 1:P], tj[:, NJ:2 * NJ], sjS[:, 0:P - 1],
                         start=False, stop=False, skip_group_check=True)
        nc.tensor.matmul(pcv[:, 0:P - 1], tj[:, 2 * NJ:3 * NJ], sjS[:, 1:P],
                         start=False, stop=False, skip_group_check=True)
        nc.tensor.matmul(pcv[:, :], tj[:, 3 * NJ:4 * NJ], smS[:, :],
                         start=False, stop=False, skip_group_check=True)
        nc.tensor.matmul(pcv[:, 1:P], tj[:, 4 * NJ:5 * NJ], smS[:, 0:P - 1],
                         start=False, stop=False, skip_group_check=True)
        nc.tensor.matmul(pcv[:, 0:P - 1], tj[:, 5 * NJ:6 * NJ], smS[:, 1:P],
                         start=False, stop=True, skip_group_check=True)
        sigT = spool.tile([NJ, P], BF16, tag="sigT", name="sigT")
        nc.scalar.activation(sigT[:], pcv[:], AF.Sigmoid)
        psig = mtp[:, 2 * P:2 * P + NJ]
        nc.tensor.transpose(psig[:], sigT[:], ident16[0:NJ, 0:NJ])
        sig_sb = spool.tile([P, NJ], BF16, tag="sig_sb", name="sig_sb")
        nc.scalar.copy(sig_sb[:], psig[:])

        # ---- ACT: squares late (feed the NEXT iteration's combine) ----
        for hq in range(2):
            sq = sqpool.tile([P, NJ * C // 2], BF16, tag="sq")
            nc.scalar.activation(sq[:], xb[:, HALF * hq:HALF * (hq + 1)],
                                 AF.Square)
            sqs.append(sq)

        if lastprev is not None:
            emit_final(lastprev)
        prev = (att, sig_sb, sqs, b)

    emit_final(prev)


def _build_nc(reps=1):
    nc = bacc.Bacc("TRN2", target_bir_lowering=False, debug=False,
                   enable_asserts=False, num_devices=N_CORES)
    x_d = nc.dram_tensor("xin", [NB, L, C], FP32, kind="ExternalInput").ap()
    w1_d = nc.dram_tensor("w1sb", [P, 2 * HB], FP32, kind="ExternalInput").ap()
    b1_d = nc.dram_tensor("b1col", [HB, 1], FP32, kind="ExternalInput").ap()
    w2b_d = nc.dram_tensor("w2b", [HB, C], FP32, kind="ExternalInput").ap()
    id_d = nc.dram_tensor("ident", [P, P], FP32, kind="ExternalInput").ap()
    id16_d = nc.dram_tensor("ident16", [P, P], BF16, kind="ExternalInput").ap()
    ones_d = nc.dram_tensor("ones", [P, P], FP32, kind="ExternalInput").ap()
    rc16_d = nc.dram_tensor("redcol16", [P, 1], BF16, kind="ExternalInput").ap()
    tj_d = nc.dram_tensor("tjconv", [NJ, 6 * NJ], FP32, kind="ExternalInput").ap()
    out_d = nc.dram_tensor("out", [NB, L, C], FP32, kind="ExternalOutput").ap()

    with tile.TileContext(nc) as tc:
        with ExitStack() as ctx:
            _build_body(ctx, tc, out_d, x_d, w1_d, b1_d, w2b_d, id_d,
                        id16_d, ones_d, rc16_d, tj_d, reps=reps)
    nc.compile()
    return nc


def get_nc(reps=1):
    key = f"nc{reps}"
    if key not in _CACHE:
        _CACHE[key] = _build_nc(reps=reps)
    return _CACHE[key]


def _prep_inputs(W1, b1, W2, b2, conv_w):
    """Host-side parameter preprocessing (shared across cores)."""
    W1 = np.asarray(W1, np.float32)
    W2 = np.asarray(W2, np.float32)
    b1 = np.asarray(b1, np.float32)
    b2 = np.asarray(b2, np.float32)
    conv_w = np.asarray(conv_w, np.float32)

    w1sb = np.zeros((P, 2 * HB), np.float32)
    for h in range(2):
        w1sb[:, HB * h:HB * h + HID] = W1[P * h:P * (h + 1), :]
    w2b = np.concatenate([W2, b2[None, :]], axis=0).astype(np.float32)
    b1col = np.concatenate([b1, [1.0]]).astype(np.float32).reshape(HB, 1)

    # Transposed-space conv Toeplitz lhsTs [j', j]; the avg tap folds in the
    # 1/C spatial-mean scale (device computes raw channel sums).
    wa = (conv_w[:, 0, 0] / C).astype(np.float32)
    wm = conv_w[:, 1, 0].astype(np.float32)
    tj = np.zeros((NJ, 6 * NJ), np.float32)
    for jp in range(NJ):
        for j in range(NJ):
            k = jp - j + 3          # main band
            if 0 <= k < 7:
                tj[jp, j] = wa[k]
                tj[jp, 3 * NJ + j] = wm[k]
            k = jp - j - 29         # prev-partition corner
            if 0 <= k < 7 and jp >= 29 and j <= 2:
                tj[jp, NJ + j] = wa[k]
                tj[jp, 4 * NJ + j] = wm[k]
            k = jp + 35 - j         # next-partition corner
            if 0 <= k < 7 and jp <= 2 and j >= 29:
                tj[jp, 2 * NJ + j] = wa[k]
                tj[jp, 5 * NJ + j] = wm[k]

    import ml_dtypes
    bf16 = ml_dtypes.bfloat16
    return {
        "w1sb": w1sb,
        "b1col": np.ascontiguousarray(b1col),
        "w2b": w2b,
        "ident": np.eye(P, dtype=np.float32),
        "ident16": np.eye(P, dtype=bf16),
        "ones": np.ones((P, P), np.float32),
        "redcol16": np.full((P, 1), 1.0 / L, bf16),
        "tjconv": tj,
    }


def kernel(x, W1, b1, W2, b2, conv_w):
    nc = get_nc()
    x = np.asarray(x, np.float32)
    params = _prep_inputs(W1, b1, W2, b2, conv_w)
    in_maps = []
    for c in range(N_CORES):
        m = dict(params)
        m["xin"] = np.ascontiguousarray(x[NB * c:NB * (c + 1)])
        in_maps.append(m)
    _CACHE["last_in_maps"] = in_maps
    res = run_bass_kernel_spmd(nc, in_maps, list(range(N_CORES)))
    _CACHE["last_results"] = res
    return np.concatenate([res.results[c]["out"] for c in range(N_CORES)],
                          axis=0)


def _pjrt_exec(nc, in_maps, n_warm=2, n_time=8):
    """Build a sharded jit for nc, run it, return (best_wall_s, result)."""
    import time
    import jax
    import concourse.mybir as mybir_
    from concourse.bass2jax import (_bass_exec_p, install_neuronx_cc_hook,
                                    partition_id_tensor)
    from jax.experimental.shard_map import shard_map
    from jax.sharding import Mesh, PartitionSpec

    install_neuronx_cc_hook()
    partition_name = (nc.partition_id_tensor.name
                      if nc.partition_id_tensor else None)
    in_names, out_names, out_avals = [], [], []
    for alloc in nc.m.functions[0].allocations:
        if not isinstance(alloc, mybir_.MemoryLocationSet):
            continue
        name = alloc.memorylocations[0].name
        if alloc.kind == "ExternalInput":
            if name != partition_name:
                in_names.append(name)
        elif alloc.kind == "ExternalOutput":
            out_names.append(name)
            out_avals.append(jax.core.ShapedArray(
                tuple(alloc.tensor_shape), mybir_.dt.np(alloc.dtype)))
    n_params = len(in_names)
    all_in_names = list(in_names) + list(out_names)
    if partition_name is not None:
        all_in_names.append(partition_name)

    def _body(*args):
        operands = list(args)
        if partition_name is not None:
            operands.append(partition_id_tensor())
        return tuple(_bass_exec_p.bind(
            *operands,
            out_avals=tuple(out_avals),
            in_names=tuple(all_in_names),
            out_names=tuple(out_names),
            lowering_input_output_aliases=(),
            sim_require_finite=True,
            sim_require_nnan=True,
            nc=nc,
        ))

    devices = jax.devices()[:N_CORES]
    mesh = Mesh(np.asarray(devices), ("core",))
    nin = n_params + len(out_names)
    sharding = jax.sharding.NamedSharding(mesh, PartitionSpec("core"))
    fn = jax.jit(shard_map(
        _body, mesh=mesh,
        in_specs=(PartitionSpec("core"),) * nin,
        out_specs=(PartitionSpec("core"),) * len(out_names),
        check_rep=False))
    dev_args = [
        jax.device_put(np.concatenate(
            [np.asarray(in_maps[c][nm]) for c in range(N_CORES)], axis=0),
            sharding)
        for nm in in_names
    ]
    for av in out_avals:
        z = np.zeros((N_CORES * av.shape[0], *av.shape[1:]), av.dtype)
        dev_args.append(jax.device_put(z, sharding))

    for _ in range(n_warm):
        out = fn(*dev_args)
        jax.block_until_ready(out)
    best = float("inf")
    for _ in range(n_time):
        t0 = time.perf_counter()
        out = fn(*dev_args)
        jax.block_until_ready(out)
        best = min(best, time.perf_counter() - t0)
    result = np.asarray(out[0]).reshape(N_CORES * NB, L, C)
    return best, result


def bench_repeat(reps=8, n_time=10, in_maps=None):
    """Isolate device exec time: time a module doing the work `reps` times
    in-kernel vs once; slope = steady-state HW time per execution."""
    if in_maps is None:
        in_maps = _CACHE["last_in_maps"]
    t1, _ = _pjrt_exec(get_nc(1), in_maps, n_time=n_time)
    tr, result = _pjrt_exec(get_nc(reps), in_maps, n_time=n_time)
    per_exec_ns = (tr - t1) / (reps - 1) * 1e9
    return per_exec_ns, result, t1 * 1e9, tr * 1e9


def bench(n_iters=30, in_maps=None):
    """Time back-to-back NEFF executions with device-resident inputs."""
    import time
    import jax
    import concourse.mybir as mybir_
    from concourse.bass2jax import (_bass_exec_p, install_neuronx_cc_hook,
                                    partition_id_tensor)
    from jax.experimental.shard_map import shard_map
    from jax.sharding import Mesh, PartitionSpec

    nc = get_nc()
    if in_maps is None:
        in_maps = _CACHE["last_in_maps"]
    install_neuronx_cc_hook()

    partition_name = (nc.partition_id_tensor.name
                      if nc.partition_id_tensor else None)
    in_names, out_names, out_avals, zero_outs = [], [], [], []
    for alloc in nc.m.functions[0].allocations:
        if not isinstance(alloc, mybir_.MemoryLocationSet):
            continue
        name = alloc.memorylocations[0].name
        if alloc.kind == "ExternalInput":
            if name != partition_name:
                in_names.append(name)
        elif alloc.kind == "ExternalOutput":
            shape = tuple(alloc.tensor_shape)
            dtype = mybir_.dt.np(alloc.dtype)
            out_names.append(name)
            out_avals.append(jax.core.ShapedArray(shape, dtype))
            zero_outs.append(np.zeros(shape, dtype))
    n_params = len(in_names)
    all_in_names = list(in_names) + list(out_names)
    if partition_name is not None:
        all_in_names.append(partition_name)

    def _body(*args):
        operands = list(args)
        if partition_name is not None:
            operands.append(partition_id_tensor())
        return tuple(_bass_exec_p.bind(
            *operands,
            out_avals=tuple(out_avals),
            in_names=tuple(all_in_names),
            out_names=tuple(out_names),
            lowering_input_output_aliases=(),
            sim_require_finite=True,
            sim_require_nnan=True,
            nc=nc,
        ))

    devices = jax.devices()[:N_CORES]
    mesh = Mesh(np.asarray(devices), ("core",))
    nin = n_params + len(out_names)
    sharded = jax.jit(shard_map(
        _body, mesh=mesh,
        in_specs=(PartitionSpec("core"),) * nin,
        out_specs=(PartitionSpec("core"),) * len(out_names),
        check_rep=False))

    concat_in = [
        np.concatenate([np.asarray(in_maps[c][nm]) for c in range(N_CORES)],
                       axis=0)
        for nm in in_names
    ]
    concat_zeros = [
        np.zeros((N_CORES * z.shape[0], *z.shape[1:]), z.dtype)
        for z in zero_outs
    ]
    sharding = jax.sharding.NamedSharding(mesh, PartitionSpec("core"))
    dev_args = [jax.device_put(a, sharding) for a in concat_in + concat_zeros]

    out = sharded(*dev_args)
    jax.block_until_ready(out)
    t0 = time.perf_counter()
    for _ in range(n_iters):
        out = sharded(*dev_args)
    jax.block_until_ready(out)
    t1 = time.perf_counter()
    per_iter_ns = (t1 - t0) / n_iters * 1e9
    result = np.asarray(out[0]).reshape(N_CORES * NB, L, C)
    return per_iter_ns, result


# revision 24
# speedup vs baseline: 1.0215x; 1.0215x over previous
"""CBAM kernel for Trainium2, 8-way batch-parallel SPMD.

Computes out = x^2 * (att_c[b,c] + sigmoid(conv(spatial_stats))[b,l]) where
att_c = sigmoid(mlp(mean_L x) + mlp(max_L x)), matching the CBAM reference.

Layout per core: 4 batches; each batch x[4096, 256] lives in SBUF as one
[128, 8192] bf16 tensor (partition p holds DRAM rows [32p, 32p+32), i.e.
32 KiB contiguous fp32 DRAM per partition; the fp32->bf16 cast happens for
free inside the SWDGE load DMA). l = 32*p + j, free col = 256*j + c.

Engine split per batch (~23 us DMA window):
  DVE  : channel-max + spatial-max bf16 fold trees (2x perf mode), 8-j
         spatial-sum tree, the 32 final (att+sig)*x^2 STTs (deferred one
         batch so they fill the next load window), small reduces
  ACT  : 24-j spatial-sum accumulator copies, squares (bf16), sigmoids,
         PSUM->SBUF copies
  PE   : channel-sum (16x [128,512] bf16 matmuls), stats transposes, MLP,
         7-tap conv as banded-Toeplitz matmuls in transposed [j, p] space
         (corner taps = column-shifted rhs, no halo exchange needed)
  POOL : SWDGE DMA queue only - cast loads, cast stores (bf16 -> fp32)
"""

import numpy as np
from contextlib import ExitStack

import concourse.bacc as bacc
import concourse.bass as bass
import concourse.tile as tile
import concourse.mybir as mybir
from concourse.bass_utils import run_bass_kernel_spmd

AF = mybir.ActivationFunctionType
ALU = mybir.AluOpType
AX = mybir.AxisListType
FP32 = mybir.dt.float32
BF16 = mybir.dt.bfloat16

N_CORES = 8
B_FULL = 32
NB = B_FULL // N_CORES  # batches per core = 4
L = 4096
C = 256
HID = 16
HB = HID + 1
P = 128
NJ = L // P  # 32 j-blocks (rows per partition)
HJ = NJ // 2  # 16 j-blocks per half
HALF = HJ * C  # 4096 free columns per half

_CACHE: dict = {}


def _build_body(ctx: ExitStack, tc, out_d, x_d, w1_d, b1_d, w2b_d, id_d,
                id16_d, ones_d, rc16_d, tj_d, reps=1):
    nc = tc.nc

    const = ctx.enter_context(tc.tile_pool(name="const", bufs=1))
    bpool = ctx.enter_context(tc.tile_pool(name="xb16", bufs=3))
    opool = ctx.enter_context(tc.tile_pool(name="outt", bufs=2))
    sqpool = ctx.enter_context(tc.tile_pool(name="sq", bufs=4))
    mpool = ctx.enter_context(tc.tile_pool(name="maxtree", bufs=1))
    smpool = ctx.enter_context(tc.tile_pool(name="spattree", bufs=1))
    spool = ctx.enter_context(tc.tile_pool(name="stats", bufs=2))
    dpool = ctx.enter_context(tc.tile_pool(name="dummy", bufs=2))
    pacc = ctx.enter_context(tc.tile_pool(name="pacc", bufs=2, space="PSUM"))
    pwk = ctx.enter_context(tc.tile_pool(name="pwk", bufs=2, space="PSUM"))
    pcnv = ctx.enter_context(tc.tile_pool(name="pcnv", bufs=2, space="PSUM"))
    pb16 = ctx.enter_context(tc.tile_pool(name="pb16", bufs=2, space="PSUM"))

    w1 = const.tile([P, 2 * HB], FP32)
    nc.sync.dma_start(w1[:], w1_d[:])
    b1 = const.tile([HB, 1], FP32)
    nc.sync.dma_start(b1[:], b1_d[:])
    w2b = const.tile([HB, C], FP32)
    nc.sync.dma_start(w2b[:], w2b_d[:])
    ident = const.tile([P, P], FP32)
    nc.sync.dma_start(ident[:], id_d[:])
    ident16 = const.tile([P, P], BF16)
    nc.sync.dma_start(ident16[:], id16_d[:])
    ones = const.tile([P, P], FP32)
    nc.sync.dma_start(ones[:], ones_d[:])
    redcol16 = const.tile([P, 1], BF16)
    nc.sync.dma_start(redcol16[:], rc16_d[:])
    tj = const.tile([NJ, 6 * NJ], FP32)
    nc.sync.dma_start(tj[:], tj_d[:])

    # Warm the ACT function tables and the DVE LUT during the first load's
    # fill shadow - otherwise they load lazily on batch 0's critical chain.
    warm = const.tile([HB, 2], FP32)
    nc.scalar.activation(warm[:], ones[0:HB, 0:2], AF.Sigmoid)
    nc.scalar.activation(warm[:], ones[0:HB, 0:2], AF.Square)
    nc.scalar.activation(warm[:], ones[0:HB, 0:2], AF.Relu, bias=b1[:])
    warm2 = const.tile([1, 1], FP32)
    nc.vector.tensor_reduce(warm2[:], ones[0:1, 0:32], axis=AX.X, op=ALU.max)

    NSA = 20  # j-blocks whose spatial sum rides ACT accumulator copies

    def emit_final(prev):
        """Final combine (att + sig) * x^2 for the previous batch + stores.

        Emitted at the top of the next iteration so the 32 DVE STTs fill
        the load window of the current batch, and the stores land on the
        SWDGE queue right behind the current batch's loads."""
        att, psig, sqs, pb = prev
        ot = opool.tile([P, NJ * C], FP32, tag="ot", name="ot")
        for j in range(NJ):
            jh = j % HJ
            nc.vector.scalar_tensor_tensor(ot[:, C * j:C * (j + 1)],
                                           att[:], psig[:, j:j + 1],
                                           sqs[j // HJ][:, C * jh:C * (jh + 1)],
                                           op0=ALU.add, op1=ALU.mult)
        ov = out_d[pb, :, :].rearrange("(p q) c -> p (q c)", p=P)
        QW = HALF // 2
        for q4 in range(4):
            nc.sync.dma_start(ov[:, QW * q4:QW * (q4 + 1)],
                              ot[:, QW * q4:QW * (q4 + 1)])

    prev = None
    seq = [b for _ in range(reps) for b in range(NB)]
    for it, b in enumerate(seq):
        last = it == len(seq) - 1
        xb = bpool.tile([P, NJ * C], BF16, tag="xb", name="xb")
        xv = x_d[b, :, :].rearrange("(p q) c -> p (q c)", p=P)
        QW = HALF // 2
        for q4 in range(4):
            nc.gpsimd.dma_start(xb[:, QW * q4:QW * (q4 + 1)],
                                xv[:, QW * q4:QW * (q4 + 1)])

        # On the last iteration the current batch's stats chain is the
        # critical path to the final stores - emit it before the previous
        # batch's (already-gated) combine so DVE prioritises it.
        if prev is not None and not last:
            emit_final(prev)
        lastprev = prev if last else None

        sej = spool.tile([P, NJ], FP32, tag="sej", name="sej")
        semf = spool.tile([P, NJ], FP32, tag="semf", name="semf")
        pcs = pacc.tile([1, 2 * C], FP32, tag="pcs")
        sqs = []
        for h in range(2):
            # ---- ACT: spatial-sum accumulator copies (j < NSA) ----
            for jh in range(HJ):
                j = HJ * h + jh
                if j < NSA:
                    dummy = dpool.tile([P, C], BF16, tag="dummy")
                    nc.scalar.activation(dummy[:], xb[:, C * j:C * (j + 1)],
                                         AF.Identity,
                                         accum_out=sej[:, j:j + 1])
            # ---- PE: channel sum, 8x [128, 512] bf16 matmuls per half ----
            for m8 in range(8):
                m = 8 * h + m8
                nc.tensor.matmul(pcs[:], redcol16[:],
                                 xb[:, 512 * m:512 * (m + 1)],
                                 start=(m == 0), stop=(m == 15),
                                 skip_group_check=True)

        # ---- DVE: channel max over j, flat bf16 fold tree ----
        mh = mpool.tile([P, NJ * C // 2], BF16, tag="mh", name="mh")
        nc.vector.tensor_max(mh[:], xb[:, 0:HALF], xb[:, HALF:2 * HALF])
        w = HALF // 2
        while w >= C:
            nc.vector.tensor_max(mh[:, 0:w], mh[:, 0:w], mh[:, w:2 * w])
            w //= 2

        # ---- DVE: spatial max over c, strided bf16 fold tree ----
        sm = smpool.tile([P, NJ * (C // 2)], BF16, tag="sm", name="sm")
        sm3 = sm[:].rearrange("p (j c) -> p j c", c=C // 2)
        v3 = xb[:].rearrange("p (j c) -> p j c", c=C)
        nc.vector.tensor_max(sm3[:, :, :], v3[:, :, 0:C // 2],
                             v3[:, :, C // 2:C])
        w = C // 4
        while w >= 1:
            out = (semf[:, :].rearrange("p (j o) -> p j o", o=1)
                   if w == 1 else sm3[:, :, 0:w])
            nc.vector.tensor_max(out, sm3[:, :, 0:w], sm3[:, :, w:2 * w])
            w //= 2

        # ---- DVE: spatial sum for j >= NSA, small bf16 add tree ----
        nrem = NJ - NSA
        ss = smpool.tile([P, nrem * (C // 2)], BF16, tag="ss", name="ss")
        ss3 = ss[:].rearrange("p (j c) -> p j c", c=C // 2)
        v8 = xb[:, C * NSA:C * NJ].rearrange("p (j c) -> p j c", c=C)
        nc.vector.tensor_add(ss3[:, :, :], v8[:, :, 0:C // 2],
                             v8[:, :, C // 2:C])
        w = C // 4
        while w >= 1:
            out = (sej[:, NSA:NJ].rearrange("p (j o) -> p j o", o=1)
                   if w == 1 else ss3[:, :, 0:w])
            nc.vector.tensor_add(out, ss3[:, :, 0:w], ss3[:, :, w:2 * w])
            w //= 2

        # ---- channel stats into c-major [128, 4] via PE transposes ----
        avgw = spool.tile([1, 2 * C], FP32, tag="avgw", name="avgw")
        nc.scalar.copy(avgw[:], pcs[0:1, :])
        avg_row = spool.tile([1, C], FP32, tag="avg", name="avg")
        nc.vector.tensor_add(avg_row[:], avgw[0:1, 0:C], avgw[0:1, C:2 * C])
        stats = spool.tile([P, 4], FP32, tag="stats", name="stats")
        # fp32 PSUM bank: po 0:256, ph 256:258, pT 258:260
        wk = pwk.tile([P, C + 4], FP32, tag="wk")
        pT = wk[:, C + 2:C + 4]
        nc.tensor.transpose(pT[:, 0:1], avg_row[0:1, 0:P], ident[0:1, 0:1])
        nc.tensor.transpose(pT[:, 1:2], avg_row[0:1, P:C], ident[0:1, 0:1])
        # bf16 PSUM bank: mT 0:256, sig 256:288
        mtp = pb16.tile([P, 2 * P + NJ], BF16, tag="mtp")
        mT = mtp[:, 0:2 * P]
        nc.tensor.transpose(mT[:, 0:P], mh[:, 0:P], ident16[:, :])
        nc.tensor.transpose(mT[:, P:2 * P], mh[:, P:C], ident16[:, :])
        nc.scalar.copy(stats[:, 0:1], pT[:, 0:1])
        nc.scalar.copy(stats[:, 2:3], pT[:, 1:2])
        nc.vector.tensor_reduce(stats[:, 1:2], mT[:, 0:P],
                                axis=AX.X, op=ALU.max)
        nc.vector.tensor_reduce(stats[:, 3:4], mT[:, P:2 * P],
                                axis=AX.X, op=ALU.max)

        # ---- shared MLP: row HID carries the 2*b2 constant trick ----
        ph = wk[0:HB, C:C + 2]
        nc.tensor.matmul(ph[:], w1[:, 0:HB], stats[:, 0:2],
                         start=True, stop=False, skip_group_check=True)
        nc.tensor.matmul(ph[:], w1[:, HB:2 * HB], stats[:, 2:4],
                         start=False, stop=True, skip_group_check=True)
        hsb = spool.tile([HB, 2], FP32, tag="hsb", name="hsb")
        nc.scalar.activation(hsb[:], ph[:], AF.Relu, bias=b1[:])
        h2 = spool.tile([HB, 1], FP32, tag="h2", name="h2")
        nc.vector.tensor_add(h2[:], hsb[:, 0:1], hsb[:, 1:2])
        h2r = spool.tile([HB, P], FP32, tag="h2r", name="h2r")
        nc.scalar.mul(h2r[:], ones[0:HB, :], h2[:])
        po = wk[:, 0:C]
        nc.tensor.matmul(po[:], h2r[:], w2b[:], start=True, stop=True,
                         skip_group_check=True)
        att = spool.tile([P, C], BF16, tag="att", name="att")
        nc.scalar.activation(att[:], po[:], AF.Sigmoid)

        # ---- 7-tap conv in transposed [j, p] space (PE Toeplitz) ----
        # fp32 PSUM bank: sjT 0:128, smT 128:256, pcv 256:384
        pct = pcnv.tile([NJ, 3 * P], FP32, tag="pct")
        nc.tensor.transpose(pct[:, 0:P], sej[:], ident[:, :])
        nc.tensor.transpose(pct[:, P:2 * P], semf[:], ident[:, :])
        sjS = spool.tile([NJ, P], FP32, tag="sjS", name="sjS")
        nc.scalar.copy(sjS[:], pct[:, 0:P])
        smS = spool.tile([NJ, P], FP32, tag="smS", name="smS")
        nc.scalar.copy(smS[:], pct[:, P:2 * P])
        pcv = pct[:, 2 * P:3 * P]
        nc.tensor.matmul(pcv[:, :], tj[:, 0:NJ], sjS[:, :],
                         start=True, stop=False, skip_group_check=True)
        nc.tensor.matmul(pcv[:, 1:P], tj[:, NJ:2 * NJ], sjS[:, 0:P - 1],
                         start=False, stop=False, skip_group_check=True)
        nc.tensor.matmul(pcv[:, 0:P - 1], tj[:, 2 * NJ:3 * NJ], sjS[:, 1:P],
                         start=False, stop=False, skip_group_check=True)
        nc.tensor.matmul(pcv[:, :], tj[:, 3 * NJ:4 * NJ], smS[:, :],
                         start=False, stop=False, skip_group_check=True)
        nc.tensor.matmul(pcv[:, 1:P], tj[:, 4 * NJ:5 * NJ], smS[:, 0:P - 1],
                         start=False, stop=False, skip_group_check=True)
        nc.tensor.matmul(pcv[:, 0:P - 1], tj[:, 5 * NJ:6 * NJ], smS[:, 1:P],
                         start=False, stop=True, skip_group_check=True)
        sigT = spool.tile([NJ, P], BF16, tag="sigT", name="sigT")
        nc.scalar.activation(sigT[:], pcv[:], AF.Sigmoid)
        psig = mtp[:, 2 * P:2 * P + NJ]
        nc.tensor.transpose(psig[:], sigT[:], ident16[0:NJ, 0:NJ])
        sig_sb = spool.tile([P, NJ], BF16, tag="sig_sb", name="sig_sb")
        nc.scalar.copy(sig_sb[:], psig[:])

        # ---- ACT: squares late (feed the NEXT iteration's combine) ----
        for hq in range(2):
            sq = sqpool.tile([P, NJ * C // 2], BF16, tag="sq")
            nc.scalar.activation(sq[:], xb[:, HALF * hq:HALF * (hq + 1)],
                                 AF.Square)
            sqs.append(sq)

        if lastprev is not None:
            emit_final(lastprev)
        prev = (att, sig_sb, sqs, b)

    emit_final(prev)


def _build_nc(reps=1):
    nc = bacc.Bacc("TRN2", target_bir_lowering=False, debug=False,
                   enable_asserts=False, num_devices=N_CORES)
    x_d = nc.dram_tensor("xin", [NB, L, C], FP32, kind="ExternalInput").ap()
    w1_d = nc.dram_tensor("w1sb", [P, 2 * HB], FP32, kind="ExternalInput").ap()
    b1_d = nc.dram_tensor("b1col", [HB, 1], FP32, kind="ExternalInput").ap()
    w2b_d = nc.dram_tensor("w2b", [HB, C], FP32, kind="ExternalInput").ap()
    id_d = nc.dram_tensor("ident", [P, P], FP32, kind="ExternalInput").ap()
    id16_d = nc.dram_tensor("ident16", [P, P], BF16, kind="ExternalInput").ap()
    ones_d = nc.dram_tensor("ones", [P, P], FP32, kind="ExternalInput").ap()
    rc16_d = nc.dram_tensor("redcol16", [P, 1], BF16, kind="ExternalInput").ap()
    tj_d = nc.dram_tensor("tjconv", [NJ, 6 * NJ], FP32, kind="ExternalInput").ap()
    out_d = nc.dram_tensor("out", [NB, L, C], FP32, kind="ExternalOutput").ap()

    with tile.TileContext(nc) as tc:
        with ExitStack() as ctx:
            _build_body(ctx, tc, out_d, x_d, w1_d, b1_d, w2b_d, id_d,
                        id16_d, ones_d, rc16_d, tj_d, reps=reps)
    nc.compile()
    return nc


def get_nc(reps=1):
    key = f"nc{reps}"
    if key not in _CACHE:
        _CACHE[key] = _build_nc(reps=reps)
    return _CACHE[key]


def _prep_inputs(W1, b1, W2, b2, conv_w):
    """Host-side parameter preprocessing (shared across cores)."""
    W1 = np.asarray(W1, np.float32)
    W2 = np.asarray(W2, np.float32)
    b1 = np.asarray(b1, np.float32)
    b2 = np.asarray(b2, np.float32)
    conv_w = np.asarray(conv_w, np.float32)

    w1sb = np.zeros((P, 2 * HB), np.float32)
    for h in range(2):
        w1sb[:, HB * h:HB * h + HID] = W1[P * h:P * (h + 1), :]
    w2b = np.concatenate([W2, b2[None, :]], axis=0).astype(np.float32)
    b1col = np.concatenate([b1, [1.0]]).astype(np.float32).reshape(HB, 1)

    # Transposed-space conv Toeplitz lhsTs [j', j]; the avg tap folds in the
    # 1/C spatial-mean scale (device computes raw channel sums).
    wa = (conv_w[:, 0, 0] / C).astype(np.float32)
    wm = conv_w[:, 1, 0].astype(np.float32)
    tj = np.zeros((NJ, 6 * NJ), np.float32)
    for jp in range(NJ):
        for j in range(NJ):
            k = jp - j + 3          # main band
            if 0 <= k < 7:
                tj[jp, j] = wa[k]
                tj[jp, 3 * NJ + j] = wm[k]
            k = jp - j - 29         # prev-partition corner
            if 0 <= k < 7 and jp >= 29 and j <= 2:
                tj[jp, NJ + j] = wa[k]
                tj[jp, 4 * NJ + j] = wm[k]
            k = jp + 35 - j         # next-partition corner
            if 0 <= k < 7 and jp <= 2 and j >= 29:
                tj[jp, 2 * NJ + j] = wa[k]
                tj[jp, 5 * NJ + j] = wm[k]

    import ml_dtypes
    bf16 = ml_dtypes.bfloat16
    return {
        "w1sb": w1sb,
        "b1col": np.ascontiguousarray(b1col),
        "w2b": w2b,
        "ident": np.eye(P, dtype=np.float32),
        "ident16": np.eye(P, dtype=bf16),
        "ones": np.ones((P, P), np.float32),
        "redcol16": np.full((P, 1), 1.0 / L, bf16),
        "tjconv": tj,
    }


def kernel(x, W1, b1, W2, b2, conv_w):
    nc = get_nc()
    x = np.asarray(x, np.float32)
    params = _prep_inputs(W1, b1, W2, b2, conv_w)
    in_maps = []
    for c in range(N_CORES):
        m = dict(params)
        m["xin"] = np.ascontiguousarray(x[NB * c:NB * (c + 1)])
        in_maps.append(m)
    _CACHE["last_in_maps"] = in_maps
    res = run_bass_kernel_spmd(nc, in_maps, list(range(N_CORES)))
    _CACHE["last_results"] = res
    return np.concatenate([res.results[c]["out"] for c in range(N_CORES)],
                          axis=0)


def _pjrt_exec(nc, in_maps, n_warm=2, n_time=8):
    """Build a sharded jit for nc, run it, return (best_wall_s, result)."""
    import time
    import jax
    import concourse.mybir as mybir_
    from concourse.bass2jax import (_bass_exec_p, install_neuronx_cc_hook,
                                    partition_id_tensor)
    from jax.experimental.shard_map import shard_map
    from jax.sharding import Mesh, PartitionSpec

    install_neuronx_cc_hook()
    partition_name = (nc.partition_id_tensor.name
                      if nc.partition_id_tensor else None)
    in_names, out_names, out_avals = [], [], []
    for alloc in nc.m.functions[0].allocations:
        if not isinstance(alloc, mybir_.MemoryLocationSet):
            continue
        name = alloc.memorylocations[0].name
        if alloc.kind == "ExternalInput":
            if name != partition_name:
                in_names.append(name)
        elif alloc.kind == "ExternalOutput":
            out_names.append(name)
            out_avals.append(jax.core.ShapedArray(
                tuple(alloc.tensor_shape), mybir_.dt.np(alloc.dtype)))
    n_params = len(in_names)
    all_in_names = list(in_names) + list(out_names)
    if partition_name is not None:
        all_in_names.append(partition_name)

    def _body(*args):
        operands = list(args)
        if partition_name is not None:
            operands.append(partition_id_tensor())
        return tuple(_bass_exec_p.bind(
            *operands,
            out_avals=tuple(out_avals),
            in_names=tuple(all_in_names),
            out_names=tuple(out_names),
            lowering_input_output_aliases=(),
            sim_require_finite=True,
            sim_require_nnan=True,
            nc=nc,
        ))

    devices = jax.devices()[:N_CORES]
    mesh = Mesh(np.asarray(devices), ("core",))
    nin = n_params + len(out_names)
    sharding = jax.sharding.NamedSharding(mesh, PartitionSpec("core"))
    fn = jax.jit(shard_map(
        _body, mesh=mesh,
        in_specs=(PartitionSpec("core"),) * nin,
        out_specs=(PartitionSpec("core"),) * len(out_names),
        check_rep=False))
    dev_args = [
        jax.device_put(np.concatenate(
            [np.asarray(in_maps[c][nm]) for c in range(N_CORES)], axis=0),
            sharding)
        for nm in in_names
    ]
    for av in out_avals:
        z = np.zeros((N_CORES * av.shape[0], *av.shape[1:]), av.dtype)
        dev_args.append(jax.device_put(z, sharding))

    for _ in range(n_warm):
        out = fn(*dev_args)
        jax.block_until_ready(out)
    best = float("inf")
    for _ in range(n_time):
        t0 = time.perf_counter()
        out = fn(*dev_args)
        jax.block_until_ready(out)
        best = min(best, time.perf_counter() - t0)
    result = np.asarray(out[0]).reshape(N_CORES * NB, L, C)
    return best, result


def bench_repeat(reps=8, n_time=10, in_maps=None):
    """Isolate device exec time: time a module doing the work `reps` times
    in-kernel vs once; slope = steady-state HW time per execution."""
    if in_maps is None:
        in_maps = _CACHE["last_in_maps"]
    t1, _ = _pjrt_exec(get_nc(1), in_maps, n_time=n_time)
    tr, result = _pjrt_exec(get_nc(reps), in_maps, n_time=n_time)
    per_exec_ns = (tr - t1) / (reps - 1) * 1e9
    return per_exec_ns, result, t1 * 1e9, tr * 1e9


def bench(n_iters=30, in_maps=None):
    """Time back-to-back NEFF executions with device-resident inputs."""
    import time
    import jax
    import concourse.mybir as mybir_
    from concourse.bass2jax import (_bass_exec_p, install_neuronx_cc_hook,
                                    partition_id_tensor)
    from jax.experimental.shard_map import shard_map
    from jax.sharding import Mesh, PartitionSpec

    nc = get_nc()
    if in_maps is None:
        in_maps = _CACHE["last_in_maps"]
    install_neuronx_cc_hook()

    partition_name = (nc.partition_id_tensor.name
                      if nc.partition_id_tensor else None)
    in_names, out_names, out_avals, zero_outs = [], [], [], []
    for alloc in nc.m.functions[0].allocations:
        if not isinstance(alloc, mybir_.MemoryLocationSet):
            continue
        name = alloc.memorylocations[0].name
        if alloc.kind == "ExternalInput":
            if name != partition_name:
                in_names.append(name)
        elif alloc.kind == "ExternalOutput":
            shape = tuple(alloc.tensor_shape)
            dtype = mybir_.dt.np(alloc.dtype)
            out_names.append(name)
            out_avals.append(jax.core.ShapedArray(shape, dtype))
            zero_outs.append(np.zeros(shape, dtype))
    n_params = len(in_names)
    all_in_names = list(in_names) + list(out_names)
    if partition_name is not None:
        all_in_names.append(partition_name)

    def _body(*args):
        operands = list(args)
        if partition_name is not None:
            operands.append(partition_id_tensor())
        return tuple(_bass_exec_p.bind(
            *operands,
            out_avals=tuple(out_avals),
            in_names=tuple(all_in_names),
            out_names=tuple(out_names),
            lowering_input_output_aliases=(),
            sim_require_finite=True,
            sim_require_nnan=True,
            nc=nc,
        ))

    devices = jax.devices()[:N_CORES]
    mesh = Mesh(np.asarray(devices), ("core",))
    nin = n_params + len(out_names)
    sharded = jax.jit(shard_map(
        _body, mesh=mesh,
        in_specs=(PartitionSpec("core"),) * nin,
        out_specs=(PartitionSpec("core"),) * len(out_names),
        check_rep=False))

    concat_in = [
        np.concatenate([np.asarray(in_maps[c][nm]) for c in range(N_CORES)],
                       axis=0)
        for nm in in_names
    ]
    concat_zeros = [
        np.zeros((N_CORES * z.shape[0], *z.shape[1:]), z.dtype)
        for z in zero_outs
    ]
    sharding = jax.sharding.NamedSharding(mesh, PartitionSpec("core"))
    dev_args = [jax.device_put(a, sharding) for a in concat_in + concat_zeros]

    out = sharded(*dev_args)
    jax.block_until_ready(out)
    t0 = time.perf_counter()
    for _ in range(n_iters):
        out = sharded(*dev_args)
    jax.block_until_ready(out)
    t1 = time.perf_counter()
    per_iter_ns = (t1 - t0) / n_iters * 1e9
    result = np.asarray(out[0]).reshape(N_CORES * NB, L, C)
    return per_iter_ns, result
